# revision 17
# baseline (speedup 1.0000x reference)
"""GQA attention (B=2, S=2048, 16 Q heads / 8 KV heads, head_dim=128, RoPE,
no causal mask) on 8 Trainium2 NeuronCores.

Sharding: DP=2 on batch x TP=4 on heads. Each core computes 4 Q heads /
2 KV heads for one batch element, plus a row-sharded o_proj partial; the
host sums the 4 partials per batch (the "all-reduce").

v3: software-pipelined weave.
 - softmax denominator moved off the PE: DVE pairwise tree over the 16
   exp tiles + a single 2-wide ones-matmul per unit (PE -123k cycles)
 - reciprocal as Exp(-Ln(x)) on ACT (ln/exp/copy share one act table)
 - attention units interleaved at matmul granularity with Q-proj chunks
   and o_proj (st,nn) tiles as PE spacer work, so the ACT exp chain
   never throttles the PE
 - V-proj evictions on ACT (idle during pre-phase), o_proj evictions on
   DVE, output stored bf16 (host upcasts + sums partials in f32)
 - x loaded in 16 pieces on the SP DMA queue, weights on the ACT queue
"""
import json
import math
from contextlib import ExitStack

import numpy as np

# ---------------------------------------------------------------------------
# Environment patches (required for the walrus build in this container)
# ---------------------------------------------------------------------------
_PATCHED = False


def _install_patches():
    """1) The walrus here rejects >1 sync wait per instruction; split extra
    waits onto single-wait NoOps inserted before the instruction (engines
    execute their stream in order, so semantics are preserved).
    2) antenv.axon_hooks is missing in this image; shim it so trace=True
    profiling works (used by test harnesses; harmless otherwise)."""
    global _PATCHED
    if _PATCHED:
        return
    _PATCHED = True

    import concourse.bass as bass

    counter = [0]

    def _split_multiwait(bir):
        for func in bir.get("functions", []):
            for block in func.get("blocks", []):
                new_insts = []
                for inst in block.get("instructions", []):
                    si = inst.get("sync_info")
                    waits = (si or {}).get("on_wait") or []
                    if len(waits) > 1:
                        for w in waits[:-1]:
                            counter[0] += 1
                            new_insts.append(
                                {
                                    "debug": inst.get("debug", 0),
                                    "engine": inst.get("engine"),
                                    "ins": [],
                                    "name": f"I-waitsplit-{counter[0]}",
                                    "opcode": "NoOp",
                                    "outs": [],
                                    "sync_info": {"on_wait": [w], "on_update": []},
                                }
                            )
                        si["on_wait"] = [waits[-1]]
                    new_insts.append(inst)
                block["instructions"] = new_insts
        return bir

    orig_to_json_bytes = bass.Bass.to_json_bytes

    def patched_to_json_bytes(self):
        bir = json.loads(orig_to_json_bytes(self))
        return json.dumps(_split_multiwait(bir)).encode()

    bass.Bass.to_json_bytes = patched_to_json_bytes

    # -- NTFF profile hook shim (for trace=True) --
    import sys
    import types

    if "antenv.axon_hooks" not in sys.modules:
        mod = types.ModuleType("antenv.axon_hooks")
        _hook = [None]
        try:
            from trn_agent_boot.trn_boot import _ntff_profile_via_ctypes

            _hook[0] = _ntff_profile_via_ctypes("/opt/axon/libaxon_pjrt.so")
        except Exception:
            pass
        mod.get_axon_ntff_profile_hook = lambda: _hook[0]
        mod.set_axon_ntff_profile_hook = lambda h: _hook.__setitem__(0, h)
        sys.modules["antenv.axon_hooks"] = mod

    # upload_artifacts needs external storage; make it a no-op locally.
    import concourse.bass_utils as bu

    bu.upload_artifacts = lambda tmpdir: str(tmpdir)


# ---------------------------------------------------------------------------
# Problem constants (hardcoded per contest contract)
# ---------------------------------------------------------------------------
B, S, HID = 2, 2048, 2048
N_HEADS, N_KV = 16, 8
HD = 128
TP = 4  # tensor-parallel factor over heads
NQ = N_HEADS // TP  # 4 q heads per core
NKV = N_KV // TP  # 2 kv heads per core
KT = HID // 128  # 16 contraction tiles
ST = S // 128  # 16 sequence tiles of 128
SC = 512  # free-dim chunk
NB = S // SC  # 4 chunks over S
SCALE = 1.0 / math.sqrt(HD)


def _build_nc():
    import concourse.bass as bass
    import concourse.tile as tile
    from concourse import mybir

    f32 = mybir.dt.float32
    bf16 = mybir.dt.bfloat16
    AF = mybir.ActivationFunctionType

    nc = bass.Bass()
    xT = nc.dram_tensor("xT", [HID, S], bf16, kind="ExternalInput")
    wq = nc.dram_tensor("wq", [HID, NQ * HD], bf16, kind="ExternalInput")
    wk = nc.dram_tensor("wk", [HID, NKV * HD], bf16, kind="ExternalInput")
    wv = nc.dram_tensor("wv", [HID, NKV * HD], bf16, kind="ExternalInput")
    wo = nc.dram_tensor("wo", [NQ * HD, HID], bf16, kind="ExternalInput")
    cos2 = nc.dram_tensor("cos2", [HD // 2, S], bf16, kind="ExternalInput")
    sin2 = nc.dram_tensor("sin2", [HD // 2, S], bf16, kind="ExternalInput")
    out = nc.dram_tensor("out", [S, HID], bf16, kind="ExternalOutput")

    with tile.TileContext(nc) as tc, ExitStack() as ctx:
        # ---- pools ----
        const = ctx.enter_context(tc.tile_pool(name="const", bufs=1))
        keep = ctx.enter_context(tc.tile_pool(name="keep", bufs=1))
        # PSUM banks: psmm 3 + pspv 2 + psq 1 + psden 1 + psop 1 = 8
        psmm = ctx.enter_context(tc.tile_pool(name="psmm", bufs=2, space="PSUM"))
        pspv = ctx.enter_context(tc.tile_pool(name="pspv", bufs=1, space="PSUM"))
        psq = ctx.enter_context(tc.tile_pool(name="psq", bufs=1, space="PSUM"))
        psden = ctx.enter_context(tc.tile_pool(name="psden", bufs=1, space="PSUM"))
        psop = ctx.enter_context(tc.tile_pool(name="psop", bufs=1, space="PSUM"))
        rstage = ctx.enter_context(tc.tile_pool(name="rstage", bufs=1))
        ptpool = ctx.enter_context(tc.tile_pool(name="ptpool", bufs=1))
        treep = ctx.enter_context(tc.tile_pool(name="treep", bufs=1))
        sbtmp = ctx.enter_context(tc.tile_pool(name="sbtmp", bufs=2))
        ostage_pool = ctx.enter_context(tc.tile_pool(name="ostage", bufs=2))

        ones_f = const.tile([128, 128], f32)
        nc.vector.memset(ones_f[:], 1.0)
        ones_mat = const.tile([128, 128], bf16)
        nc.vector.tensor_copy(ones_mat[:], ones_f[:])

        # persistent SBUF tensors
        kT_sb = [keep.tile([128, S], bf16, tag=f"kT{g}", name=f"kT{g}") for g in range(NKV)]
        qT_sb = [keep.tile([128, 2, SC], bf16, tag=f"qT{h}", name=f"qT{h}") for h in range(NQ)]
        v_sb = keep.tile([128, ST, NKV * HD], bf16, tag="v", name="v")
        oh_sb = [keep.tile([128, 2, SC], bf16, tag=f"oh{h}", name=f"oh{h}") for h in range(NQ)]
        x_sb = keep.tile([128, KT, S], bf16, tag="x")
        wq_sb = keep.tile([128, KT, NQ * HD], bf16, tag="wq")
        wo_sb = keep.tile([128, NQ, HID], bf16, tag="wo")
        cos_sb = keep.tile([HD // 2, S], bf16, tag="cos")
        sin_sb = keep.tile([HD // 2, S], bf16, tag="sin")

        # ------------- DMA loads -------------
        # One strictly-ordered SP queue matching consumption order (the
        # HBM is the bottleneck ~350GB/s; parallel queues just reorder
        # arrivals against need). wo is triggered after pre-phase emission
        # on the Pool queue (needed only from round 1).
        x_re = xT.rearrange("(kt p) s -> p kt s", p=128)
        wpre = ExitStack()
        wkvpool = wpre.enter_context(tc.tile_pool(name="wkv", bufs=1))
        wk_sb = wkvpool.tile([128, KT, NKV * HD], bf16, tag="wk")
        wk_re = wk.rearrange("(kt p) d -> p kt d", p=128)
        wq_re = wq.rearrange("(kt p) (h d) -> p kt (h d)", p=128, d=HD)
        nc.sync.dma_start(out=cos_sb[:], in_=cos2[:, :])
        nc.sync.dma_start(out=sin_sb[:], in_=sin2[:, :])
        for kq in range(4):
            nc.sync.dma_start(
                out=wk_sb[:, 4 * kq : 4 * kq + 4, :], in_=wk_re[:, 4 * kq : 4 * kq + 4, :]
            )
        for c in (0, 1):
            for kq in range(2):
                nc.sync.dma_start(
                    out=x_sb[:, 8 * kq : 8 * kq + 8, c * SC : (c + 1) * SC],
                    in_=x_re[:, 8 * kq : 8 * kq + 8, c * SC : (c + 1) * SC],
                )
        for kq in range(2):
            nc.sync.dma_start(
                out=wq_sb[:, 8 * kq : 8 * kq + 8, :], in_=wq_re[:, 8 * kq : 8 * kq + 8, :]
            )
        for c in (2, 3):
            for kq in range(2):
                nc.sync.dma_start(
                    out=x_sb[:, 8 * kq : 8 * kq + 8, c * SC : (c + 1) * SC],
                    in_=x_re[:, 8 * kq : 8 * kq + 8, c * SC : (c + 1) * SC],
                )
        wv_sb = wkvpool.tile([128, KT, NKV * HD], bf16, tag="wv")
        nc.sync.dma_start(out=wv_sb[:], in_=wv.rearrange("(kt p) d -> p kt d", p=128))

        # ---- RoPE: ps [128(re/im),512] -> dst[:, c-slice] ----
        # (PSUM inputs may cross partition bases; SBUF+SBUF may not, hence
        # the gpsimd (Pool) add/sub for the writes at partition base 0/64)
        def rope_emit(ps, dst_lo, dst_hi, c0):
            re = ps[0:64, :]
            im = ps[64:128, :]
            cs = cos_sb[:, c0 : c0 + SC]
            sn = sin_sb[:, c0 : c0 + SC]
            t1 = rstage.tile([64, SC], f32, tag="t1")
            t2 = rstage.tile([64, SC], f32, tag="t2")
            t3 = rstage.tile([64, SC], f32, tag="t3")
            t4 = rstage.tile([64, SC], f32, tag="t4")
            nc.vector.tensor_mul(t1[:], re, cs)
            nc.vector.tensor_mul(t2[:], im, sn)
            nc.vector.tensor_sub(dst_lo, t1[:], t2[:])
            nc.vector.tensor_mul(t3[:], re, sn)
            nc.vector.tensor_mul(t4[:], im, cs)
            nc.gpsimd.tensor_add(dst_hi, t3[:], t4[:])

        # ------------- emission helpers -------------
        def k_chunk(c, g):
            psp = psmm.tile([128, 2 * SC], f32, tag="mm", name=f"kps{c}_{g}")
            ps = psp[:, 0:SC]
            for kt in range(KT):
                nc.tensor.matmul(
                    ps,
                    wk_sb[:, kt, g * HD : (g + 1) * HD],
                    x_sb[:, kt, c * SC : (c + 1) * SC],
                    start=(kt == 0),
                    stop=(kt == KT - 1),
                )
            rope_emit(
                ps,
                kT_sb[g][0:64, c * SC : c * SC + SC],
                kT_sb[g][64:128, c * SC : c * SC + SC],
                c * SC,
            )

        def v_st(st):
            ps = psop.tile([128, SC], f32, tag="op", name=f"vps{st}")
            for kt in range(KT):
                nc.tensor.matmul(
                    ps[:, 0 : NKV * HD],
                    x_sb[:, kt, st * 128 : (st + 1) * 128],
                    wv_sb[:, kt, :],
                    start=(kt == 0),
                    stop=(kt == KT - 1),
                )
            nc.vector.tensor_copy(v_sb[:, st, :], ps[:, 0 : NKV * HD])

        # Q proj for (h, c): 16 matmuls split into 4 spacer pops + rope
        qps_live = {}

        def q_part(h, c, part):
            if part == 0:
                qps_live[(h, c)] = psq.tile([128, SC], f32, tag="q", name=f"qps{h}_{c}")
            ps = qps_live[(h, c)]
            for kt in range(4 * part, 4 * part + 4):
                nc.tensor.matmul(
                    ps[:],
                    wq_sb[:, kt, h * HD : (h + 1) * HD],
                    x_sb[:, kt, c * SC : (c + 1) * SC],
                    start=(kt == 0),
                    stop=(kt == KT - 1),
                )
            if part == 3:
                rope_emit(
                    ps,
                    qT_sb[h][0:64, c % 2, :],
                    qT_sb[h][64:128, c % 2, :],
                    c * SC,
                )
                del qps_live[(h, c)]

        # o_proj (st, nn): 4 accumulating matmuls + DVE evict; DMA per st
        ostage_live = {}

        def o_part(st, nn, pl=None, fine=False):
            if nn == 0:
                ostage_live[st] = ostage_pool.tile([128, S], bf16, tag="ostage", name=f"ostage{st}")
            pl = pl or psop
            pso = pl.tile([128, SC], f32, tag="op" if pl is psop else "q", name=f"ops{st}_{nn}")
            for h in range(NQ):
                nc.tensor.matmul(
                    pso[:],
                    oh_sb[h][:, (st // 4) % 2, (st % 4) * 128 : (st % 4 + 1) * 128],
                    wo_sb[:, h, nn * SC : (nn + 1) * SC],
                    start=(h == 0),
                    stop=(h == NQ - 1),
                )
            stg = ostage_live[st]
            nc.vector.tensor_copy(stg[:, nn * SC : (nn + 1) * SC], pso[:])
            if fine:
                nc.sync.dma_start(
                    out=out[st * 128 : (st + 1) * 128, nn * SC : (nn + 1) * SC],
                    in_=stg[:, nn * SC : (nn + 1) * SC],
                )
                if nn == NB - 1:
                    del ostage_live[st]
            elif nn == NB - 1:
                nc.sync.dma_start(out=out[st * 128 : (st + 1) * 128, :], in_=stg[:])
                del ostage_live[st]

        # ------------- pre-phase -------------
        # Ordered against the single DMA queue: K chunks as x lands, Q
        # heads once wq lands, V last (wv arrives at the end).
        k_chunk(0, 0)
        k_chunk(0, 1)
        k_chunk(1, 0)
        k_chunk(1, 1)
        for part in range(4):
            q_part(0, 0, part)
        k_chunk(2, 0)
        k_chunk(2, 1)
        for part in range(4):
            q_part(1, 0, part)
        k_chunk(3, 0)
        k_chunk(3, 1)
        for part in range(4):
            q_part(2, 0, part)
        v_st(0)
        v_st(1)
        for part in range(4):
            q_part(3, 0, part)
        v_st(2)
        v_st(3)
        # wo lands ~60us in, needed from round 1 only
        nc.gpsimd.dma_start(out=wo_sb[:], in_=wo.rearrange("(h p) n -> p h n", p=128))

        # ------------- attention units, woven -------------
        def emit_recip_chain(u):
            """recb = exp(-ln(den)) on ACT (ln/exp/copy share one table)."""
            lnt = sbtmp.tile([128, SC], f32, tag="lnt")
            nc.scalar.activation(lnt[:], u["psd"][:], AF.Ln)
            recb = sbtmp.tile([128, SC], f32, tag="recb")
            nc.scalar.activation(recb[:], lnt[:], AF.Exp, scale=-1.0)
            u["recb"] = recb

        def emit_norm(u):
            nc.vector.tensor_mul(
                oh_sb[u["h"]][:, u["r"] % 2, :],
                u["ppv"][:],
                u["recb"][:],
            )

        spacers = []

        def pop_spacer():
            if spacers:
                spacers.pop(0)()

        def score_pair(h, r, p):
            g = h // (NQ // NKV)
            pss = psmm.tile([128, 2 * SC], f32, tag="mm", name=f"ssp{h}_{r}_{p}")
            for j in range(2):
                kt = 2 * p + j
                nc.tensor.matmul(
                    pss[:, j * SC : (j + 1) * SC],
                    kT_sb[g][:, kt * 128 : (kt + 1) * 128],
                    qT_sb[h][:, r % 2, :],
                    start=True,
                    stop=True,
                )
            return pss

        def unit(h, r, prev, nxt, start_pops=3):
            g = h // (NQ // NKV)
            pt = ptpool.tile([128, KT, SC], bf16, tag="pt")
            me = {"h": h, "r": r}
            for _ in range(start_pops):
                pop_spacer()
            # -- paired scores + exps; p0/p1 scores may have been pre-emitted
            #    at the tail of the previous unit (PE never waits the psmm
            #    ring at a unit boundary) --
            pre = prev.pop("pre", []) if prev is not None else []
            for p in range(KT // 2):
                if p < len(pre):
                    pss = pre[p]
                else:
                    pss = score_pair(h, r, p)
                nc.scalar.activation(
                    pt[:, 2 * p : 2 * p + 2, :], pss[:], AF.Exp, scale=SCALE
                )
                if p == 2 and prev is not None:
                    emit_recip_chain(prev)
                if p == 3 and prev is not None:
                    emit_norm(prev)
                if p in (3, 5):
                    pop_spacer()
            # -- denominator tree (DVE) + 2-wide ones matmul --
            A = treep.tile([128, 8, SC], bf16, tag="A")
            Bt = treep.tile([128, 4, SC], bf16, tag="B")
            Ct = treep.tile([128, 2, SC], bf16, tag="C")
            nc.vector.tensor_add(A[:], pt[:, 0:8, :], pt[:, 8:16, :])
            nc.vector.tensor_add(Bt[:], A[:, 0:4, :], A[:, 4:8, :])
            nc.vector.tensor_add(Ct[:], Bt[:, 0:2, :], Bt[:, 2:4, :])
            psd = psden.tile([128, SC], f32, tag="den")
            for j in range(2):
                nc.tensor.matmul(
                    psd[:], ones_mat[:], Ct[:, j, :], start=(j == 0), stop=(j == 1)
                )
            me["psd"] = psd
            # -- PV + spacers --
            ppv = pspv.tile([128, SC], f32, tag="pv")
            for kt in range(KT):
                nc.tensor.matmul(
                    ppv[:],
                    v_sb[:, kt, g * HD : (g + 1) * HD],
                    pt[:, kt, :],
                    start=(kt == 0),
                    stop=(kt == KT - 1),
                )
                if kt % 3 == 2:
                    pop_spacer()
            me["ppv"] = ppv
            # -- pre-emit the next unit's first two score pairs --
            if nxt is not None:
                me["pre"] = [score_pair(nxt[0], nxt[1], p) for p in range(2)]
            return me

        # Per-round spacer schedule. o_parts of round r-1's st tiles must
        # come after enough q-pops that the norm for head 3 of round r-1
        # (emitted at kt==7 of the first unit of round r) precedes them.
        def round_spacers(r):
            sp = []
            if r == 0:
                for st in range(4, 16):
                    sp.append(lambda st=st: v_st(st))
                for h in range(NQ):
                    for part in range(4):
                        sp.append(lambda h=h, part=part: q_part(h, 1, part))
            elif r == 1:
                for h in range(NQ):
                    for part in range(4):
                        sp.append(lambda h=h, part=part: q_part(h, 2, part))
                for st in range(0, 4):
                    for nn in range(NB):
                        sp.append(lambda st=st, nn=nn: o_part(st, nn))
            elif r == 2:
                for h in range(NQ):
                    for part in range(4):
                        sp.append(lambda h=h, part=part: q_part(h, 3, part))
                for st in range(4, 6):
                    for nn in range(NB):
                        sp.append(lambda st=st, nn=nn: o_part(st, nn))
            else:
                for st in range(6, 11):
                    for nn in range(NB):
                        pl = psop if nn % 2 == 0 else psq
                        sp.append(lambda st=st, nn=nn, pl=pl: o_part(st, nn, pl))
            return sp

        prev = None
        seq = [(h, r) for r in range(NB) for h in range(NQ)]
        for i, (h, r) in enumerate(seq):
            if h == 0:
                spacers = spacers + round_spacers(r)
            nxt = seq[i + 1] if i + 1 < len(seq) else None
            prev = unit(h, r, prev, nxt, start_pops=6 if i == 0 else 3)
            if h == NQ - 1 and r == NB - 1:
                while spacers:
                    pop_spacer()

        # ------------- tail -------------
        emit_recip_chain(prev)
        for nn in range(NB):
            o_part(11, nn, psop if nn % 2 == 0 else psq)
        emit_norm(prev)
        for st in range(12, 16):
            for nn in range(NB):
                o_part(st, nn, psop if nn % 2 == 0 else psq, fine=(st >= 14))
        wpre.close()
    return nc


_NC_CACHE = None


def _get_nc():
    global _NC_CACHE
    if _NC_CACHE is None:
        _install_patches()
        _NC_CACHE = _build_nc()
    return _NC_CACHE


# De-interleave permutation: within each head, even dims then odd dims.
_PERM = np.concatenate([np.arange(0, HD, 2), np.arange(1, HD, 2)])

_last_in_maps = None


def kernel(x, Wq, Wk, Wv, Wo, freqs_cos, freqs_sin, start_pos):
    _install_patches()
    import ml_dtypes

    from concourse.bass_utils import run_bass_kernel_spmd

    bf16 = ml_dtypes.bfloat16
    x = np.asarray(x, dtype=np.float32)
    Wq = np.asarray(Wq, dtype=np.float32)
    Wk = np.asarray(Wk, dtype=np.float32)
    Wv = np.asarray(Wv, dtype=np.float32)
    Wo = np.asarray(Wo, dtype=np.float32)
    cos2 = np.ascontiguousarray(np.asarray(freqs_cos, dtype=np.float32).T).astype(bf16)
    sin2 = np.ascontiguousarray(np.asarray(freqs_sin, dtype=np.float32).T).astype(bf16)

    # Per-head de-interleave of Wq/Wk columns (RoPE pairs -> [re, im] blocks)
    Wq_p = Wq.reshape(HID, N_HEADS, HD)[:, :, _PERM]
    Wk_p = Wk.reshape(HID, N_KV, HD)[:, :, _PERM]

    in_maps = []
    for core in range(8):
        b, t = divmod(core, TP)
        xT_b = np.ascontiguousarray(x[b].T.astype(bf16))
        wq_c = np.ascontiguousarray(
            Wq_p[:, t * NQ : (t + 1) * NQ, :].reshape(HID, NQ * HD).astype(bf16)
        )
        wk_c = np.ascontiguousarray(
            Wk_p[:, t * NKV : (t + 1) * NKV, :].reshape(HID, NKV * HD).astype(bf16)
        )
        wv_c = np.ascontiguousarray(
            Wv.reshape(HID, N_KV, HD)[:, t * NKV : (t + 1) * NKV, :]
            .reshape(HID, NKV * HD)
            .astype(bf16)
        )
        wo_c = np.ascontiguousarray(
            Wo[t * NQ * HD : (t + 1) * NQ * HD, :].astype(bf16)
        )
        in_maps.append(
            {
                "xT": xT_b,
                "wq": wq_c,
                "wk": wk_c,
                "wv": wv_c,
                "wo": wo_c,
                "cos2": cos2,
                "sin2": sin2,
            }
        )

    global _last_in_maps
    _last_in_maps = in_maps
    nc = _get_nc()
    res = run_bass_kernel_spmd(nc, in_maps, list(range(8)))
    outs = [res.results[c]["out"].astype(np.float32) for c in range(8)]
    full = np.stack(
        [sum(outs[b * TP + t] for t in range(TP)) for b in range(B)]
    ).astype(np.float32)
    return full


# revision 18
# speedup vs baseline: 1.0056x; 1.0056x over previous
"""GQA attention (B=2, S=2048, 16 Q heads / 8 KV heads, head_dim=128, RoPE,
no causal mask) on 8 Trainium2 NeuronCores.

Sharding: DP=2 on batch x TP=4 on heads. Each core computes 4 Q heads /
2 KV heads for one batch element, plus a row-sharded o_proj partial; the
host sums the 4 partials per batch (the "all-reduce").

v3: software-pipelined weave.
 - softmax denominator moved off the PE: DVE pairwise tree over the 16
   exp tiles + a single 2-wide ones-matmul per unit (PE -123k cycles)
 - reciprocal as Exp(-Ln(x)) on ACT (ln/exp/copy share one act table)
 - attention units interleaved at matmul granularity with Q-proj chunks
   and o_proj (st,nn) tiles as PE spacer work, so the ACT exp chain
   never throttles the PE
 - V-proj evictions on ACT (idle during pre-phase), o_proj evictions on
   DVE, output stored bf16 (host upcasts + sums partials in f32)
 - x loaded in 16 pieces on the SP DMA queue, weights on the ACT queue
"""
import json
import math
from contextlib import ExitStack

import numpy as np

# ---------------------------------------------------------------------------
# Environment patches (required for the walrus build in this container)
# ---------------------------------------------------------------------------
_PATCHED = False


def _install_patches():
    """1) The walrus here rejects >1 sync wait per instruction; split extra
    waits onto single-wait NoOps inserted before the instruction (engines
    execute their stream in order, so semantics are preserved).
    2) antenv.axon_hooks is missing in this image; shim it so trace=True
    profiling works (used by test harnesses; harmless otherwise)."""
    global _PATCHED
    if _PATCHED:
        return
    _PATCHED = True

    import concourse.bass as bass

    counter = [0]

    def _split_multiwait(bir):
        for func in bir.get("functions", []):
            for block in func.get("blocks", []):
                new_insts = []
                for inst in block.get("instructions", []):
                    si = inst.get("sync_info")
                    waits = (si or {}).get("on_wait") or []
                    if len(waits) > 1:
                        for w in waits[:-1]:
                            counter[0] += 1
                            new_insts.append(
                                {
                                    "debug": inst.get("debug", 0),
                                    "engine": inst.get("engine"),
                                    "ins": [],
                                    "name": f"I-waitsplit-{counter[0]}",
                                    "opcode": "NoOp",
                                    "outs": [],
                                    "sync_info": {"on_wait": [w], "on_update": []},
                                }
                            )
                        si["on_wait"] = [waits[-1]]
                    new_insts.append(inst)
                block["instructions"] = new_insts
        return bir

    orig_to_json_bytes = bass.Bass.to_json_bytes

    def patched_to_json_bytes(self):
        bir = json.loads(orig_to_json_bytes(self))
        return json.dumps(_split_multiwait(bir)).encode()

    bass.Bass.to_json_bytes = patched_to_json_bytes

    # -- NTFF profile hook shim (for trace=True) --
    import sys
    import types

    if "antenv.axon_hooks" not in sys.modules:
        mod = types.ModuleType("antenv.axon_hooks")
        _hook = [None]
        try:
            from trn_agent_boot.trn_boot import _ntff_profile_via_ctypes

            _hook[0] = _ntff_profile_via_ctypes("/opt/axon/libaxon_pjrt.so")
        except Exception:
            pass
        mod.get_axon_ntff_profile_hook = lambda: _hook[0]
        mod.set_axon_ntff_profile_hook = lambda h: _hook.__setitem__(0, h)
        sys.modules["antenv.axon_hooks"] = mod

    # upload_artifacts needs external storage; make it a no-op locally.
    import concourse.bass_utils as bu

    bu.upload_artifacts = lambda tmpdir: str(tmpdir)


# ---------------------------------------------------------------------------
# Problem constants (hardcoded per contest contract)
# ---------------------------------------------------------------------------
B, S, HID = 2, 2048, 2048
N_HEADS, N_KV = 16, 8
HD = 128
TP = 4  # tensor-parallel factor over heads
NQ = N_HEADS // TP  # 4 q heads per core
NKV = N_KV // TP  # 2 kv heads per core
KT = HID // 128  # 16 contraction tiles
ST = S // 128  # 16 sequence tiles of 128
SC = 512  # free-dim chunk
NB = S // SC  # 4 chunks over S
SCALE = 1.0 / math.sqrt(HD)


def _build_nc():
    import concourse.bass as bass
    import concourse.tile as tile
    from concourse import mybir

    f32 = mybir.dt.float32
    bf16 = mybir.dt.bfloat16
    AF = mybir.ActivationFunctionType

    nc = bass.Bass()
    xT = nc.dram_tensor("xT", [HID, S], bf16, kind="ExternalInput")
    wq = nc.dram_tensor("wq", [HID, NQ * HD], bf16, kind="ExternalInput")
    wk = nc.dram_tensor("wk", [HID, NKV * HD], bf16, kind="ExternalInput")
    wv = nc.dram_tensor("wv", [HID, NKV * HD], bf16, kind="ExternalInput")
    wo = nc.dram_tensor("wo", [NQ * HD, HID], bf16, kind="ExternalInput")
    cos2 = nc.dram_tensor("cos2", [HD // 2, S], bf16, kind="ExternalInput")
    sin2 = nc.dram_tensor("sin2", [HD // 2, S], bf16, kind="ExternalInput")
    out = nc.dram_tensor("out", [S, HID], bf16, kind="ExternalOutput")

    with tile.TileContext(nc) as tc, ExitStack() as ctx:
        # ---- pools ----
        const = ctx.enter_context(tc.tile_pool(name="const", bufs=1))
        keep = ctx.enter_context(tc.tile_pool(name="keep", bufs=1))
        # PSUM banks: psmm 3 + pspv 2 + psq 1 + psden 1 + psop 1 = 8
        psmm = ctx.enter_context(tc.tile_pool(name="psmm", bufs=2, space="PSUM"))
        pspv = ctx.enter_context(tc.tile_pool(name="pspv", bufs=1, space="PSUM"))
        psq = ctx.enter_context(tc.tile_pool(name="psq", bufs=1, space="PSUM"))
        psden = ctx.enter_context(tc.tile_pool(name="psden", bufs=1, space="PSUM"))
        psop = ctx.enter_context(tc.tile_pool(name="psop", bufs=1, space="PSUM"))
        rstage = ctx.enter_context(tc.tile_pool(name="rstage", bufs=1))
        ptpool = ctx.enter_context(tc.tile_pool(name="ptpool", bufs=1))
        treep = ctx.enter_context(tc.tile_pool(name="treep", bufs=1))
        sbtmp = ctx.enter_context(tc.tile_pool(name="sbtmp", bufs=2))
        ostage_pool = ctx.enter_context(tc.tile_pool(name="ostage", bufs=2))

        ones_f = const.tile([128, 128], f32)
        nc.vector.memset(ones_f[:], 1.0)
        ones_mat = const.tile([128, 128], bf16)
        nc.vector.tensor_copy(ones_mat[:], ones_f[:])

        # persistent SBUF tensors
        kT_sb = [keep.tile([128, S], bf16, tag=f"kT{g}", name=f"kT{g}") for g in range(NKV)]
        qT_sb = [keep.tile([128, 2, SC], bf16, tag=f"qT{h}", name=f"qT{h}") for h in range(NQ)]
        v_sb = keep.tile([128, ST, NKV * HD], bf16, tag="v", name="v")
        oh_sb = [keep.tile([128, 2, SC], bf16, tag=f"oh{h}", name=f"oh{h}") for h in range(NQ)]
        x_sb = keep.tile([128, KT, S], bf16, tag="x")
        wq_sb = keep.tile([128, KT, NQ * HD], bf16, tag="wq")
        wo_sb = keep.tile([128, NQ, HID], bf16, tag="wo")
        cos_sb = keep.tile([HD // 2, S], bf16, tag="cos")
        sin_sb = keep.tile([HD // 2, S], bf16, tag="sin")

        # ------------- DMA loads -------------
        # One strictly-ordered SP queue matching consumption order (the
        # HBM is the bottleneck ~350GB/s; parallel queues just reorder
        # arrivals against need). wo is triggered after pre-phase emission
        # on the Pool queue (needed only from round 1).
        x_re = xT.rearrange("(kt p) s -> p kt s", p=128)
        wpre = ExitStack()
        wkvpool = wpre.enter_context(tc.tile_pool(name="wkv", bufs=1))
        wk_sb = wkvpool.tile([128, KT, NKV * HD], bf16, tag="wk")
        wk_re = wk.rearrange("(kt p) d -> p kt d", p=128)
        wq_re = wq.rearrange("(kt p) (h d) -> p kt (h d)", p=128, d=HD)
        nc.scalar.dma_start(out=cos_sb[:], in_=cos2[:, :])
        nc.scalar.dma_start(out=sin_sb[:], in_=sin2[:, :])
        for kq in range(4):
            nc.scalar.dma_start(
                out=wk_sb[:, 4 * kq : 4 * kq + 4, :], in_=wk_re[:, 4 * kq : 4 * kq + 4, :]
            )
        for kq in range(4):
            nc.scalar.dma_start(
                out=wq_sb[:, 4 * kq : 4 * kq + 4, :], in_=wq_re[:, 4 * kq : 4 * kq + 4, :]
            )
        wv_sb = wkvpool.tile([128, KT, NKV * HD], bf16, tag="wv")
        nc.scalar.dma_start(out=wv_sb[:], in_=wv.rearrange("(kt p) d -> p kt d", p=128))
        for c in range(NB):
            for kq in range(4):
                nc.sync.dma_start(
                    out=x_sb[:, 4 * kq : 4 * kq + 4, c * SC : (c + 1) * SC],
                    in_=x_re[:, 4 * kq : 4 * kq + 4, c * SC : (c + 1) * SC],
                )

        # ---- RoPE: ps [128(re/im),512] -> dst[:, c-slice] ----
        # (PSUM inputs may cross partition bases; SBUF+SBUF may not, hence
        # the gpsimd (Pool) add/sub for the writes at partition base 0/64)
        def rope_emit(ps, dst_lo, dst_hi, c0):
            re = ps[0:64, :]
            im = ps[64:128, :]
            cs = cos_sb[:, c0 : c0 + SC]
            sn = sin_sb[:, c0 : c0 + SC]
            t1 = rstage.tile([64, SC], f32, tag="t1")
            t2 = rstage.tile([64, SC], f32, tag="t2")
            t3 = rstage.tile([64, SC], f32, tag="t3")
            t4 = rstage.tile([64, SC], f32, tag="t4")
            nc.vector.tensor_mul(t1[:], re, cs)
            nc.vector.tensor_mul(t2[:], im, sn)
            nc.vector.tensor_sub(dst_lo, t1[:], t2[:])
            nc.vector.tensor_mul(t3[:], re, sn)
            nc.vector.tensor_mul(t4[:], im, cs)
            nc.gpsimd.tensor_add(dst_hi, t3[:], t4[:])

        # ------------- emission helpers -------------
        def k_chunk(c, g):
            psp = psmm.tile([128, 2 * SC], f32, tag="mm", name=f"kps{c}_{g}")
            ps = psp[:, 0:SC]
            for kt in range(KT):
                nc.tensor.matmul(
                    ps,
                    wk_sb[:, kt, g * HD : (g + 1) * HD],
                    x_sb[:, kt, c * SC : (c + 1) * SC],
                    start=(kt == 0),
                    stop=(kt == KT - 1),
                )
            rope_emit(
                ps,
                kT_sb[g][0:64, c * SC : c * SC + SC],
                kT_sb[g][64:128, c * SC : c * SC + SC],
                c * SC,
            )

        def v_st(st):
            ps = psop.tile([128, SC], f32, tag="op", name=f"vps{st}")
            for kt in range(KT):
                nc.tensor.matmul(
                    ps[:, 0 : NKV * HD],
                    x_sb[:, kt, st * 128 : (st + 1) * 128],
                    wv_sb[:, kt, :],
                    start=(kt == 0),
                    stop=(kt == KT - 1),
                )
            nc.vector.tensor_copy(v_sb[:, st, :], ps[:, 0 : NKV * HD])

        # Q proj for (h, c): 16 matmuls split into 4 spacer pops + rope
        qps_live = {}

        def q_part(h, c, part):
            if part == 0:
                qps_live[(h, c)] = psq.tile([128, SC], f32, tag="q", name=f"qps{h}_{c}")
            ps = qps_live[(h, c)]
            for kt in range(4 * part, 4 * part + 4):
                nc.tensor.matmul(
                    ps[:],
                    wq_sb[:, kt, h * HD : (h + 1) * HD],
                    x_sb[:, kt, c * SC : (c + 1) * SC],
                    start=(kt == 0),
                    stop=(kt == KT - 1),
                )
            if part == 3:
                rope_emit(
                    ps,
                    qT_sb[h][0:64, c % 2, :],
                    qT_sb[h][64:128, c % 2, :],
                    c * SC,
                )
                del qps_live[(h, c)]

        # o_proj (st, nn): 4 accumulating matmuls + DVE evict; DMA per st
        ostage_live = {}

        def o_part(st, nn, pl=None, fine=False):
            if nn == 0:
                ostage_live[st] = ostage_pool.tile([128, S], bf16, tag="ostage", name=f"ostage{st}")
            pl = pl or psop
            pso = pl.tile([128, SC], f32, tag="op" if pl is psop else "q", name=f"ops{st}_{nn}")
            for h in range(NQ):
                nc.tensor.matmul(
                    pso[:],
                    oh_sb[h][:, (st // 4) % 2, (st % 4) * 128 : (st % 4 + 1) * 128],
                    wo_sb[:, h, nn * SC : (nn + 1) * SC],
                    start=(h == 0),
                    stop=(h == NQ - 1),
                )
            stg = ostage_live[st]
            nc.vector.tensor_copy(stg[:, nn * SC : (nn + 1) * SC], pso[:])
            if fine:
                nc.sync.dma_start(
                    out=out[st * 128 : (st + 1) * 128, nn * SC : (nn + 1) * SC],
                    in_=stg[:, nn * SC : (nn + 1) * SC],
                )
                if nn == NB - 1:
                    del ostage_live[st]
            elif nn == NB - 1:
                nc.sync.dma_start(out=out[st * 128 : (st + 1) * 128, :], in_=stg[:])
                del ostage_live[st]

        # ------------- pre-phase -------------
        # Ordered against the single DMA queue: K chunks as x lands, Q
        # heads once wq lands, V last (wv arrives at the end).
        k_chunk(0, 0)
        k_chunk(0, 1)
        k_chunk(1, 0)
        k_chunk(1, 1)
        for part in range(4):
            q_part(0, 0, part)
        k_chunk(2, 0)
        k_chunk(2, 1)
        for part in range(4):
            q_part(1, 0, part)
        k_chunk(3, 0)
        k_chunk(3, 1)
        for part in range(4):
            q_part(2, 0, part)
        v_st(0)
        v_st(1)
        for part in range(4):
            q_part(3, 0, part)
        v_st(2)
        v_st(3)
        # wo lands ~60us in, needed from round 1 only
        nc.gpsimd.dma_start(out=wo_sb[:], in_=wo.rearrange("(h p) n -> p h n", p=128))

        # ------------- attention units, woven -------------
        def emit_recip_chain(u):
            """recb = exp(-ln(den)) on ACT (ln/exp/copy share one table)."""
            lnt = sbtmp.tile([128, SC], f32, tag="lnt")
            nc.scalar.activation(lnt[:], u["psd"][:], AF.Ln)
            recb = sbtmp.tile([128, SC], f32, tag="recb")
            nc.scalar.activation(recb[:], lnt[:], AF.Exp, scale=-1.0)
            u["recb"] = recb

        def emit_norm(u):
            nc.vector.tensor_mul(
                oh_sb[u["h"]][:, u["r"] % 2, :],
                u["ppv"][:],
                u["recb"][:],
            )

        spacers = []

        def pop_spacer():
            if spacers:
                spacers.pop(0)()

        def score_pair(h, r, p):
            g = h // (NQ // NKV)
            pss = psmm.tile([128, 2 * SC], f32, tag="mm", name=f"ssp{h}_{r}_{p}")
            for j in range(2):
                kt = 2 * p + j
                nc.tensor.matmul(
                    pss[:, j * SC : (j + 1) * SC],
                    kT_sb[g][:, kt * 128 : (kt + 1) * 128],
                    qT_sb[h][:, r % 2, :],
                    start=True,
                    stop=True,
                )
            return pss

        def unit(h, r, prev, nxt, start_pops=3):
            g = h // (NQ // NKV)
            pt = ptpool.tile([128, KT, SC], bf16, tag="pt")
            me = {"h": h, "r": r}
            for _ in range(start_pops):
                pop_spacer()
            # -- paired scores + exps; p0/p1 scores may have been pre-emitted
            #    at the tail of the previous unit (PE never waits the psmm
            #    ring at a unit boundary) --
            pre = prev.pop("pre", []) if prev is not None else []
            for p in range(KT // 2):
                if p < len(pre):
                    pss = pre[p]
                else:
                    pss = score_pair(h, r, p)
                nc.scalar.activation(
                    pt[:, 2 * p : 2 * p + 2, :], pss[:], AF.Exp, scale=SCALE
                )
                if p == 2 and prev is not None:
                    emit_recip_chain(prev)
                if p == 3 and prev is not None:
                    emit_norm(prev)
                if p in (3, 5):
                    pop_spacer()
            # -- denominator tree (DVE) + 2-wide ones matmul --
            A = treep.tile([128, 8, SC], bf16, tag="A")
            Bt = treep.tile([128, 4, SC], bf16, tag="B")
            Ct = treep.tile([128, 2, SC], bf16, tag="C")
            nc.vector.tensor_add(A[:], pt[:, 0:8, :], pt[:, 8:16, :])
            nc.vector.tensor_add(Bt[:], A[:, 0:4, :], A[:, 4:8, :])
            nc.vector.tensor_add(Ct[:], Bt[:, 0:2, :], Bt[:, 2:4, :])
            psd = psden.tile([128, SC], f32, tag="den")
            for j in range(2):
                nc.tensor.matmul(
                    psd[:], ones_mat[:], Ct[:, j, :], start=(j == 0), stop=(j == 1)
                )
            me["psd"] = psd
            # -- PV + spacers --
            ppv = pspv.tile([128, SC], f32, tag="pv")
            for kt in range(KT):
                nc.tensor.matmul(
                    ppv[:],
                    v_sb[:, kt, g * HD : (g + 1) * HD],
                    pt[:, kt, :],
                    start=(kt == 0),
                    stop=(kt == KT - 1),
                )
                if kt % 3 == 2:
                    pop_spacer()
            me["ppv"] = ppv
            # -- pre-emit the next unit's first two score pairs --
            if nxt is not None:
                me["pre"] = [score_pair(nxt[0], nxt[1], p) for p in range(2)]
            return me

        # Per-round spacer schedule. o_parts of round r-1's st tiles must
        # come after enough q-pops that the norm for head 3 of round r-1
        # (emitted at kt==7 of the first unit of round r) precedes them.
        def round_spacers(r):
            sp = []
            if r == 0:
                for st in range(4, 16):
                    sp.append(lambda st=st: v_st(st))
                for h in range(NQ):
                    for part in range(4):
                        sp.append(lambda h=h, part=part: q_part(h, 1, part))
            elif r == 1:
                for h in range(NQ):
                    for part in range(4):
                        sp.append(lambda h=h, part=part: q_part(h, 2, part))
                for st in range(0, 4):
                    for nn in range(NB):
                        sp.append(lambda st=st, nn=nn: o_part(st, nn))
            elif r == 2:
                for h in range(NQ):
                    for part in range(4):
                        sp.append(lambda h=h, part=part: q_part(h, 3, part))
                for st in range(4, 6):
                    for nn in range(NB):
                        sp.append(lambda st=st, nn=nn: o_part(st, nn))
            else:
                for st in range(6, 11):
                    for nn in range(NB):
                        pl = psop if nn % 2 == 0 else psq
                        sp.append(lambda st=st, nn=nn, pl=pl: o_part(st, nn, pl))
            return sp

        prev = None
        seq = [(h, r) for r in range(NB) for h in range(NQ)]
        for i, (h, r) in enumerate(seq):
            if h == 0:
                spacers = spacers + round_spacers(r)
            nxt = seq[i + 1] if i + 1 < len(seq) else None
            prev = unit(h, r, prev, nxt, start_pops=6 if i == 0 else 3)
            if h == NQ - 1 and r == NB - 1:
                while spacers:
                    pop_spacer()

        # ------------- tail -------------
        emit_recip_chain(prev)
        for nn in range(NB):
            o_part(11, nn, psop if nn % 2 == 0 else psq)
        emit_norm(prev)
        for st in range(12, 16):
            for nn in range(NB):
                o_part(st, nn, psop if nn % 2 == 0 else psq, fine=(st >= 14))
        wpre.close()
    return nc


_NC_CACHE = None


def _get_nc():
    global _NC_CACHE
    if _NC_CACHE is None:
        _install_patches()
        _NC_CACHE = _build_nc()
    return _NC_CACHE


# De-interleave permutation: within each head, even dims then odd dims.
_PERM = np.concatenate([np.arange(0, HD, 2), np.arange(1, HD, 2)])

_last_in_maps = None


def kernel(x, Wq, Wk, Wv, Wo, freqs_cos, freqs_sin, start_pos):
    _install_patches()
    import ml_dtypes

    from concourse.bass_utils import run_bass_kernel_spmd

    bf16 = ml_dtypes.bfloat16
    x = np.asarray(x, dtype=np.float32)
    Wq = np.asarray(Wq, dtype=np.float32)
    Wk = np.asarray(Wk, dtype=np.float32)
    Wv = np.asarray(Wv, dtype=np.float32)
    Wo = np.asarray(Wo, dtype=np.float32)
    cos2 = np.ascontiguousarray(np.asarray(freqs_cos, dtype=np.float32).T).astype(bf16)
    sin2 = np.ascontiguousarray(np.asarray(freqs_sin, dtype=np.float32).T).astype(bf16)

    # Per-head de-interleave of Wq/Wk columns (RoPE pairs -> [re, im] blocks)
    Wq_p = Wq.reshape(HID, N_HEADS, HD)[:, :, _PERM]
    Wk_p = Wk.reshape(HID, N_KV, HD)[:, :, _PERM]

    in_maps = []
    for core in range(8):
        b, t = divmod(core, TP)
        xT_b = np.ascontiguousarray(x[b].T.astype(bf16))
        wq_c = np.ascontiguousarray(
            Wq_p[:, t * NQ : (t + 1) * NQ, :].reshape(HID, NQ * HD).astype(bf16)
        )
        wk_c = np.ascontiguousarray(
            Wk_p[:, t * NKV : (t + 1) * NKV, :].reshape(HID, NKV * HD).astype(bf16)
        )
        wv_c = np.ascontiguousarray(
            Wv.reshape(HID, N_KV, HD)[:, t * NKV : (t + 1) * NKV, :]
            .reshape(HID, NKV * HD)
            .astype(bf16)
        )
        wo_c = np.ascontiguousarray(
            Wo[t * NQ * HD : (t + 1) * NQ * HD, :].astype(bf16)
        )
        in_maps.append(
            {
                "xT": xT_b,
                "wq": wq_c,
                "wk": wk_c,
                "wv": wv_c,
                "wo": wo_c,
                "cos2": cos2,
                "sin2": sin2,
            }
        )

    global _last_in_maps
    _last_in_maps = in_maps
    nc = _get_nc()
    res = run_bass_kernel_spmd(nc, in_maps, list(range(8)))
    outs = [res.results[c]["out"].astype(np.float32) for c in range(8)]
    full = np.stack(
        [sum(outs[b * TP + t] for t in range(TP)) for b in range(B)]
    ).astype(np.float32)
    return full
